# revision 17
# baseline (speedup 1.0000x reference)
"""Plastic (Hebbian) FC layer — Trainium2 Bass kernel, 8 NeuronCores. v3

Block-Jacobi formulation (see kernel_v2): per 16-step block,
  Y^0 = tanh(Z0);  Y^k = tanh(Z0 + tril(A) @_s Y^{k-1}),  K=2 sweeps
  Z0  = x@w + xtd@(alpha o G_prefix) + cross-term from previous block.

v3: the pair tensor xtd[t] (x) xq[s] (strictly-lower-masked in-half +
cross-half) is precomputed on the host and DMA'd per block, so the
A-tensor matmuls run a full block ahead of the sweeps; x@w base is
computed once for all 128 steps; the cross reduction runs on GPSIMD.
"""

import sys

for _p in ("/opt/trn_rl_repo", "/opt/pypackages"):
    if _p not in sys.path:
        sys.path.insert(0, _p)

import numpy as np
import ml_dtypes

B, T, N = 8, 128, 512
TB = 16
NB = T // TB
NG = N // 128
N_CORES = 8
NSWEEP = 2
BF16 = ml_dtypes.bfloat16


def _build(eta_f: float):
    import concourse.bass as bass
    import concourse.tile as tile
    from concourse import bacc, mybir

    f32 = mybir.dt.float32
    bfd = mybir.dt.bfloat16
    mult = mybir.AluOpType.mult
    add = mybir.AluOpType.add

    nc = bacc.Bacc(None, target_bir_lowering=False)

    xt_e = nc.declare_dram_parameter("xt", [128, NG, T], bfd, isOutput=False)
    xtd_e = nc.declare_dram_parameter("xtd", [128, NG, T], bfd, isOutput=False)
    xh_e = nc.declare_dram_parameter("xh", [TB, NB, N], bfd, isOutput=False)
    wm_e = nc.declare_dram_parameter("wm", [128, NG, N], bfd, isOutput=False)
    ab_e = nc.declare_dram_parameter("ab", [128, NG, N], bfd, isOutput=False)
    id_e = nc.declare_dram_parameter("ident", [128, 128], bfd, isOutput=False)
    pr_e = nc.declare_dram_parameter(
        "pr", [128, NB, NG, 2 * TB, TB], bfd, isOutput=False)
    yo_e = nc.declare_dram_parameter("yout", [128, NG, T], bfd, isOutput=True)

    with tile.TileContext(nc) as tc:
        with (
            tc.tile_pool(name="persist", bufs=1) as pp,
            tc.tile_pool(name="blkbuf", bufs=2) as bp,
            tc.tile_pool(name="ps_ht", bufs=1, space=bass.MemorySpace.PSUM) as ps_ht,
            tc.tile_pool(name="ps_aps", bufs=1, space=bass.MemorySpace.PSUM) as ps_aps,
            tc.tile_pool(name="ps_misc", bufs=1, space=bass.MemorySpace.PSUM) as ps_misc,
            tc.tile_pool(name="ps_bb", bufs=1, space=bass.MemorySpace.PSUM) as ps_bb,
        ):
            XT = pp.tile([128, NG, T], bfd)
            XTD = pp.tile([128, NG, T], bfd)
            XH = pp.tile([TB, NB, N], bfd)
            WM = pp.tile([128, NG, N], bfd)
            AB = pp.tile([128, NG, N], bfd)
            IDT = pp.tile([128, 128], bfd)
            WBASE = pp.tile([128, NG, T], f32)
            HTC = pp.tile([128, NG, N], bfd)
            AEFF = pp.tile([128, NG, N], bfd)
            Y = pp.tile([128, NG, T], bfd)
            YTR = pp.tile([TB, NG, 128], bfd)
            HT = ps_ht.tile([128, NG, N], f32)

            Tanh = mybir.ActivationFunctionType.Tanh
            Copy = mybir.ActivationFunctionType.Copy

            # ---- PE warm-up: harmless matmuls on a memset tile so the
            #      HAM clock-gate opens before the real work arrives ----
            DUM = pp.tile([128, 64], bfd)
            nc.vector.memset(DUM[:], 0.0)
            DPS = ps_bb.tile([64, 64], f32, tag="bb")
            for _ in range(24):
                nc.tensor.matmul(DPS[:], DUM[:, 0:64], DUM[:], start=True,
                                 stop=True, skip_group_check=True)

            # ---- input DMAs (x@w path first, then pair blocks) ----
            PRa = bp.tile([128, NG, 2 * TB, TB], bfd, tag="pr")
            PRb = bp.tile([128, NG, 2 * TB, TB], bfd, tag="pr")
            nc.sync.dma_start(PRa[:], pr_e[:, 0])
            nc.sync.dma_start(AB[:], ab_e[:])
            nc.sync.dma_start(XT[:], xt_e[:])
            nc.sync.dma_start(WM[:], wm_e[:])
            nc.sync.dma_start(PRb[:], pr_e[:, 1])
            nc.sync.dma_start(XTD[:], xtd_e[:])
            nc.sync.dma_start(XH[:], xh_e[:])
            nc.sync.dma_start(IDT[:], id_e[:])

            nc.vector.memset(Y[:], 0.0)
            _EARLY_ASB = []

            # ---- WBASE = x @ w for all 128 steps, once ----
            WB = ps_misc.tile([128, NG, T], f32, tag="misc")
            def _wbase_mms():
                for jc in range(NG):
                    for ig in range(NG):
                        nc.tensor.matmul(
                            WB[:, jc, :], WM[:, ig, jc * 128:(jc + 1) * 128],
                            XT[:, ig, :], start=(ig == 0), stop=(ig == NG - 1))
                nc.scalar.activation(WBASE[:], WB[:], Copy)

            def make_asb(PR):
                """A tensors (in-half masked + cross-half) for one block:
                16 matmuls in two jc-pair psum sets, copied to SBUF bf16."""
                ASB32 = bp.tile([128, NG, 2 * TB, TB], bfd, tag="asb")
                for jp in range(2):
                    APS = ps_aps.tile([128, 2, 2 * TB, TB], f32, tag="aps")
                    for jl in range(2):
                        jc = 2 * jp + jl
                        for ig in range(NG):
                            nc.tensor.matmul(
                                APS[:, jl, :, :],
                                AB[:, ig, jc * 128:(jc + 1) * 128],
                                PR[:, ig, :, :],
                                start=(ig == 0), stop=(ig == NG - 1))
                    nc.scalar.activation(
                        ASB32[:, 2 * jp:2 * jp + 2, :, :], APS[:], Copy)
                return ASB32

            def ybc(Ysrc, tw):
                return Ysrc[:].unsqueeze(2).broadcast_to((128, NG, tw, TB))

            def tree_reduce(eng, AYP, CONTRIB, tag, tw):
                """CONTRIB[:, :, :tw] = sum_s AYP[:, :, :tw, s] (tiles)."""
                A8 = bp.tile([128, NG, 2 * TB, 8], bfd, tag=tag + "8")
                A4 = bp.tile([128, NG, 2 * TB, 4], bfd, tag=tag + "4")
                A2 = bp.tile([128, NG, 2 * TB, 2], bfd, tag=tag + "2")
                eng.tensor_add(A8[:, :, :tw, :], AYP[:, :, :tw, 0:8],
                               AYP[:, :, :tw, 8:16])
                eng.tensor_add(A4[:, :, :tw, :], A8[:, :, :tw, 0:4],
                               A8[:, :, :tw, 4:8])
                eng.tensor_add(A2[:, :, :tw, :], A4[:, :, :tw, 0:2],
                               A4[:, :, :tw, 2:4])
                eng.tensor_add(CONTRIB[:, :, :tw], A2[:, :, :tw, 0],
                               A2[:, :, :tw, 1])

            def tree_reduce2(eng, AYP, CONTRIB, tag):
                A8 = bp.tile([128, NG, TB, 8], bfd, tag=tag + "8")
                A4 = bp.tile([128, NG, TB, 4], bfd, tag=tag + "4")
                A2 = bp.tile([128, NG, TB, 2], bfd, tag=tag + "2")
                s = slice(TB, 2 * TB)
                eng.tensor_add(A8[:], AYP[:, :, s, 0:8], AYP[:, :, s, 8:16])
                eng.tensor_add(A4[:], A8[:, :, :, 0:4], A8[:, :, :, 4:8])
                eng.tensor_add(A2[:], A4[:, :, :, 0:2], A4[:, :, :, 2:4])
                eng.tensor_add(CONTRIB[:, :, s], A2[:, :, :, 0],
                               A2[:, :, :, 1])

            ASB = make_asb(PRa)        # block 0 (leads on PE: gates sweep 1)
            _wbase_mms()               # x@w base (only gates act0)
            PRn = PRb                  # pair data for block 1
            z_src = WBASE[:, :, 0:TB]
            Y0_prev = None
            for blk in range(NB):
                t0 = blk * TB
                t1 = t0 + TB
                last_blk = blk == NB - 1

                # -- PE first: transposes of Y0(blk-1) — the first-sweep
                #    estimate, available a block earlier than the final Y,
                #    so the hebb chain de-stalls (accuracy-neutral) --
                if blk >= 1:
                    YTP = ps_misc.tile([TB, NG, 128], bfd, tag="misc")
                    for jc in range(NG):
                        nc.tensor.transpose(
                            YTP[:, jc, :], Y0_prev[:, jc, :], IDT[:])

                # -- sweep: Y0 = tanh(Z0); one fused 32-row mul+tree
                #    computes both the in-block contribution (t rows 0:16)
                #    and the cross contribution to blk+1 (rows 16:32) --
                # critical path: in-block half only; the cross half (rows
                # 16:32, feeds only the next block's base) runs after.
                with tc.high_priority(400):
                    Y0 = bp.tile([128, NG, TB], bfd, tag="y0")
                    nc.scalar.activation(Y0[:], z_src, Tanh)
                    AYP = bp.tile([128, NG, 2 * TB, TB], bfd, tag="ayp")
                    nc.vector.tensor_mul(
                        AYP[:, :, 0:TB, :], ASB[:, :, 0:TB, :], ybc(Y0, TB))
                    CT = bp.tile([128, NG, 2 * TB], f32, tag="ctr")
                    tree_reduce(nc.vector, AYP, CT, "ay", TB)
                    Z1 = bp.tile([128, NG, TB], f32, tag="z")
                    nc.vector.tensor_add(Z1[:], CT[:, :, 0:TB], z_src)
                    nc.scalar.activation(Y[:, :, t0:t1], Z1[:], Tanh)
                if not last_blk:
                    nc.vector.tensor_mul(
                        AYP[:, :, TB:2 * TB, :], ASB[:, :, TB:2 * TB, :],
                        ybc(Y0, TB))
                    tree_reduce2(nc.vector, AYP, CT, "ax")

                nc.sync.dma_start(yo_e[:, :, t0:t1], Y[:, :, t0:t1])
                Y0_prev = Y0
                if last_blk:
                    break

                CC = CT[:, :, TB:2 * TB]     # cross contribution slice

                # -- A tensors for block blk+1 (PE + ScE early) --
                ASBn = make_asb(PRn)

                # -- hebb prefix: YTR copy, HT matmuls, AEFF --
                if blk >= 1:
                    nc.scalar.activation(YTR[:], YTP[:], Copy)
                    for ic in range(NG):
                        nc.tensor.matmul(
                            HT[:, ic, :],
                            XH[:, blk - 1, ic * 128:(ic + 1) * 128],
                            YTR[:, :, :],
                            start=(blk == 1), stop=(blk == NB - 2),
                            skip_group_check=True)
                    nc.scalar.activation(HTC[:], HT[:], Copy)
                    nc.vector.tensor_mul(AEFF[:], AB[:], HTC[:])

                # -- aeff part of base for block blk+1 --
                BB = None
                if blk >= 1:
                    BB = ps_bb.tile([128, NG, TB], f32, tag="bb")
                    k = 0
                    for jc in range(NG):
                        for ig in range(NG):
                            nc.tensor.matmul(
                                BB[:, jc, :],
                                AEFF[:, ig, jc * 128:(jc + 1) * 128],
                                XTD[:, ig, t1:t1 + TB],
                                start=(k == 0), stop=(k == NG * NG - 1),
                                skip_group_check=True)
                            k += 1

                # -- assemble Z0(blk+1) = WBASE + cross (+ BB) --
                CW = bp.tile([128, NG, TB], f32, tag="cw")
                nc.vector.scalar_tensor_tensor(
                    CW[:], CC, 1.0, WBASE[:, :, t1:t1 + TB], mult, add)
                if BB is not None:
                    Z0 = bp.tile([128, NG, TB], f32, tag="z0")
                    nc.vector.scalar_tensor_tensor(
                        Z0[:], BB[:], 1.0, CW[:], mult, add)
                    z_src = Z0[:]
                else:
                    z_src = CW[:]

                # prefetch pair data for block blk+2
                if blk + 2 < NB:
                    PRn = bp.tile([128, NG, 2 * TB, TB], bfd, tag="pr")
                    nc.sync.dma_start(PRn[:], pr_e[:, blk + 2])
                ASB = ASBn

    nc.compile()
    return nc


def kernel(x, w, alpha, eta, _trace=False, _trace_kwargs=None):
    from concourse.bass_utils import run_bass_kernel_spmd

    x = np.asarray(x, np.float32)
    w = np.asarray(w, np.float32)
    alpha = np.asarray(alpha, np.float32)
    eta_f = float(np.asarray(eta).reshape(-1)[0])

    d = 1.0 - eta_f
    t_idx = np.arange(T, dtype=np.float64)
    wscale = (d ** t_idx).astype(np.float32)                   # d^t
    qscale = (eta_f * d ** (-1.0 - t_idx)).astype(np.float32)  # eta*d^(-1-s)

    def to_grp(m, dt=BF16):  # [T,N] (cols=i) -> [128, NG, T], i = ig*128+ip
        return np.ascontiguousarray(
            m.T.reshape(NG, 128, T).transpose(1, 0, 2)).astype(dt)

    def to_wgrp(m, dt=BF16):  # [N,N] -> [128, NG, N], i = ig*128+ip
        return np.ascontiguousarray(
            m.reshape(NG, 128, N).transpose(1, 0, 2)).astype(dt)

    wm = to_wgrp(w)
    ab = to_wgrp(alpha)
    ident = np.eye(128, dtype=np.float32).astype(BF16)

    # pair tensor: pr[ip, b, ig, tl, s] = xtd[i, b*TB + tl] * xq[i, b*TB + s]
    # tl in [0,TB): in-block half, strictly-lower-masked (s < tl)
    # tl in [TB,2TB): cross half (t' = b*TB + TB + (tl-TB)), unmasked
    tril = np.tril(np.ones((TB, TB), np.float32), -1)

    in_maps = []
    for b_i in range(B):
        xb = x[b_i]                                 # [T, N]
        xtd = (xb * wscale[:, None]).astype(BF16).astype(np.float32)
        xq = (xb * qscale[:, None]).astype(BF16).astype(np.float32)
        pr = np.zeros((128, NB, NG, 2 * TB, TB), np.float32)
        for blk in range(NB):
            t0 = blk * TB
            xtd_in = xtd[t0:t0 + TB]                # [TB, N]
            xq_b = xq[t0:t0 + TB]                   # [TB, N]
            pin = np.einsum('tn,sn,ts->nts', xtd_in, xq_b, tril)
            g = pin.reshape(NG, 128, TB, TB).transpose(1, 0, 2, 3)
            pr[:, blk, :, 0:TB, :] = g
            if blk < NB - 1:
                xtd_c = xtd[t0 + TB:t0 + 2 * TB]
                pc = np.einsum('tn,sn->nts', xtd_c, xq_b)
                pr[:, blk, :, TB:2 * TB, :] = \
                    pc.reshape(NG, 128, TB, TB).transpose(1, 0, 2, 3)
        in_maps.append({
            "xt": to_grp(xb),
            "xtd": to_grp(xb * wscale[:, None]),
            "xh": np.ascontiguousarray(
                (xb * qscale[:, None]).reshape(NB, TB, N)
                .transpose(1, 0, 2)).astype(BF16),
            "wm": wm, "ab": ab, "ident": ident,
            "pr": np.ascontiguousarray(pr).astype(BF16),
        })

    nc = _build(eta_f)
    res = run_bass_kernel_spmd(
        nc, in_maps, list(range(N_CORES)),
        trace=_trace, **(_trace_kwargs or {}))

    out = np.empty((B, T, N), np.float32)
    for b_i in range(B):
        yo = res.results[b_i]["yout"]               # [128, NG, T] bf16
        out[b_i] = yo.astype(np.float32).transpose(2, 1, 0).reshape(T, N)
    if _trace:
        kernel.last_result = res
    return out



# revision 18
# speedup vs baseline: 1.0442x; 1.0442x over previous
"""Plastic (Hebbian) FC layer — Trainium2 Bass kernel, 8 NeuronCores. v3

Block-Jacobi formulation (see kernel_v2): per 16-step block,
  Y^0 = tanh(Z0);  Y^k = tanh(Z0 + tril(A) @_s Y^{k-1}),  K=2 sweeps
  Z0  = x@w + xtd@(alpha o G_prefix) + cross-term from previous block.

v3: the pair tensor xtd[t] (x) xq[s] (strictly-lower-masked in-half +
cross-half) is precomputed on the host and DMA'd per block, so the
A-tensor matmuls run a full block ahead of the sweeps; x@w base is
computed once for all 128 steps; the cross reduction runs on GPSIMD.
"""

import sys

for _p in ("/opt/trn_rl_repo", "/opt/pypackages"):
    if _p not in sys.path:
        sys.path.insert(0, _p)

import numpy as np
import ml_dtypes

B, T, N = 8, 128, 512
TB = 16
NB = T // TB
NG = N // 128
N_CORES = 8
NSWEEP = 2
BF16 = ml_dtypes.bfloat16


def _build(eta_f: float):
    import concourse.bass as bass
    import concourse.tile as tile
    from concourse import bacc, mybir

    f32 = mybir.dt.float32
    bfd = mybir.dt.bfloat16
    mult = mybir.AluOpType.mult
    add = mybir.AluOpType.add

    nc = bacc.Bacc(None, target_bir_lowering=False)

    xt_e = nc.declare_dram_parameter("xt", [128, NG, T], bfd, isOutput=False)
    xtd_e = nc.declare_dram_parameter("xtd", [128, NG, T], bfd, isOutput=False)
    xh_e = nc.declare_dram_parameter("xh", [TB, NB, N], bfd, isOutput=False)
    wm_e = nc.declare_dram_parameter("wm", [128, NG, N], bfd, isOutput=False)
    ab_e = nc.declare_dram_parameter("ab", [128, NG, N], bfd, isOutput=False)
    id_e = nc.declare_dram_parameter("ident", [128, 128], bfd, isOutput=False)
    pr_e = nc.declare_dram_parameter(
        "pr", [128, NB, NG, 2 * TB, TB], bfd, isOutput=False)
    yo_e = nc.declare_dram_parameter("yout", [128, NG, T], bfd, isOutput=True)

    with tile.TileContext(nc) as tc:
        with (
            tc.tile_pool(name="persist", bufs=1) as pp,
            tc.tile_pool(name="blkbuf", bufs=2) as bp,
            tc.tile_pool(name="ps_ht", bufs=1, space=bass.MemorySpace.PSUM) as ps_ht,
            tc.tile_pool(name="ps_aps", bufs=1, space=bass.MemorySpace.PSUM) as ps_aps,
            tc.tile_pool(name="ps_misc", bufs=1, space=bass.MemorySpace.PSUM) as ps_misc,
            tc.tile_pool(name="ps_bb", bufs=1, space=bass.MemorySpace.PSUM) as ps_bb,
        ):
            XT = pp.tile([128, NG, T], bfd)
            XTD = pp.tile([128, NG, T], bfd)
            XH = pp.tile([TB, NB, N], bfd)
            WM = pp.tile([128, NG, N], bfd)
            AB = pp.tile([128, NG, N], bfd)
            IDT = pp.tile([128, 128], bfd)
            WBASE = pp.tile([128, NG, T], f32)
            HTC = pp.tile([128, NG, N], bfd)
            AEFF = pp.tile([128, NG, N], bfd)
            Y = pp.tile([128, NG, T], bfd)
            YTR = pp.tile([TB, NG, 128], bfd)
            HT = ps_ht.tile([128, NG, N], f32)

            Tanh = mybir.ActivationFunctionType.Tanh
            Copy = mybir.ActivationFunctionType.Copy

            # ---- PE warm-up: harmless matmuls on a memset tile so the
            #      HAM clock-gate opens before the real work arrives ----
            DUM = pp.tile([128, 64], bfd)
            nc.vector.memset(DUM[:], 0.0)
            DPS = ps_bb.tile([64, 64], f32, tag="bb")
            for _ in range(24):
                nc.tensor.matmul(DPS[:], DUM[:, 0:64], DUM[:], start=True,
                                 stop=True, skip_group_check=True)

            # ---- input DMAs (x@w path first, then pair blocks) ----
            PRa = bp.tile([128, NG, 2 * TB, TB], bfd, tag="pr")
            PRb = bp.tile([128, NG, 2 * TB, TB], bfd, tag="pr")
            nc.sync.dma_start(PRa[:], pr_e[:, 0])
            nc.sync.dma_start(AB[:], ab_e[:])
            nc.sync.dma_start(XT[:], xt_e[:])
            nc.sync.dma_start(WM[:], wm_e[:])
            nc.sync.dma_start(PRb[:], pr_e[:, 1])
            nc.sync.dma_start(XTD[:], xtd_e[:])
            nc.sync.dma_start(XH[:], xh_e[:])
            nc.sync.dma_start(IDT[:], id_e[:])

            nc.vector.memset(Y[:], 0.0)
            _EARLY_ASB = []

            # ---- WBASE = x @ w for all 128 steps, once ----
            WB = ps_misc.tile([128, NG, T], f32, tag="misc")
            def _wbase_mms():
                for jc in range(NG):
                    for ig in range(NG):
                        nc.tensor.matmul(
                            WB[:, jc, :], WM[:, ig, jc * 128:(jc + 1) * 128],
                            XT[:, ig, :], start=(ig == 0), stop=(ig == NG - 1))
                nc.scalar.activation(WBASE[:], WB[:], Copy)

            def make_asb(PR):
                """A tensors (in-half masked + cross-half) for one block:
                16 matmuls in two jc-pair psum sets, copied to SBUF bf16."""
                ASB32 = bp.tile([128, NG, 2 * TB, TB], bfd, tag="asb")
                for jp in range(2):
                    APS = ps_aps.tile([128, 2, 2 * TB, TB], f32, tag="aps")
                    for jl in range(2):
                        jc = 2 * jp + jl
                        for ig in range(NG):
                            nc.tensor.matmul(
                                APS[:, jl, :, :],
                                AB[:, ig, jc * 128:(jc + 1) * 128],
                                PR[:, ig, :, :],
                                start=(ig == 0), stop=(ig == NG - 1))
                    nc.scalar.activation(
                        ASB32[:, 2 * jp:2 * jp + 2, :, :], APS[:], Copy)
                return ASB32

            def ybc(Ysrc, tw):
                return Ysrc[:].unsqueeze(2).broadcast_to((128, NG, tw, TB))

            def tree_reduce(eng, AYP, CONTRIB, tag, tw):
                """CONTRIB[:, :, :tw] = sum_s AYP[:, :, :tw, s] (tiles)."""
                A8 = bp.tile([128, NG, 2 * TB, 8], bfd, tag=tag + "8")
                A4 = bp.tile([128, NG, 2 * TB, 4], bfd, tag=tag + "4")
                A2 = bp.tile([128, NG, 2 * TB, 2], bfd, tag=tag + "2")
                eng.tensor_add(A8[:, :, :tw, :], AYP[:, :, :tw, 0:8],
                               AYP[:, :, :tw, 8:16])
                eng.tensor_add(A4[:, :, :tw, :], A8[:, :, :tw, 0:4],
                               A8[:, :, :tw, 4:8])
                eng.tensor_add(A2[:, :, :tw, :], A4[:, :, :tw, 0:2],
                               A4[:, :, :tw, 2:4])
                eng.tensor_add(CONTRIB[:, :, :tw], A2[:, :, :tw, 0],
                               A2[:, :, :tw, 1])

            def tree_reduce2(eng, AYP, CONTRIB, tag):
                A8 = bp.tile([128, NG, TB, 8], bfd, tag=tag + "8")
                A4 = bp.tile([128, NG, TB, 4], bfd, tag=tag + "4")
                A2 = bp.tile([128, NG, TB, 2], bfd, tag=tag + "2")
                s = slice(TB, 2 * TB)
                eng.tensor_add(A8[:], AYP[:, :, s, 0:8], AYP[:, :, s, 8:16])
                eng.tensor_add(A4[:], A8[:, :, :, 0:4], A8[:, :, :, 4:8])
                eng.tensor_add(A2[:], A4[:, :, :, 0:2], A4[:, :, :, 2:4])
                eng.tensor_add(CONTRIB[:, :, s], A2[:, :, :, 0],
                               A2[:, :, :, 1])

            ASB = make_asb(PRa)        # block 0 (leads on PE: gates sweep 1)
            _wbase_mms()               # x@w base (only gates act0)
            PRn = PRb                  # pair data for block 1
            z_src = WBASE[:, :, 0:TB]
            for blk in range(NB):
                t0 = blk * TB
                t1 = t0 + TB
                last_blk = blk == NB - 1

                # -- PE first: transposes of Y(blk-1) --
                if blk >= 1:
                    YTP = ps_misc.tile([TB, NG, 128], bfd, tag="misc")
                    for jc in range(NG):
                        nc.tensor.transpose(
                            YTP[:, jc, :], Y[:, jc, t0 - TB:t0], IDT[:])

                # -- sweep: Y0 = tanh(Z0); one fused 32-row mul+tree
                #    computes both the in-block contribution (t rows 0:16)
                #    and the cross contribution to blk+1 (rows 16:32) --
                # critical path: in-block half only; the cross half (rows
                # 16:32, feeds only the next block's base) runs after.
                with tc.high_priority(400):
                    Y0 = bp.tile([128, NG, TB], bfd, tag="y0")
                    nc.scalar.activation(Y0[:], z_src, Tanh)
                    AYP = bp.tile([128, NG, 2 * TB, TB], bfd, tag="ayp")
                    nc.vector.tensor_mul(
                        AYP[:, :, 0:TB, :], ASB[:, :, 0:TB, :], ybc(Y0, TB))
                    CT = bp.tile([128, NG, 2 * TB], f32, tag="ctr")
                    tree_reduce(nc.vector, AYP, CT, "ay", TB)
                    Z1 = bp.tile([128, NG, TB], f32, tag="z")
                    nc.vector.tensor_add(Z1[:], CT[:, :, 0:TB], z_src)
                    nc.scalar.activation(Y[:, :, t0:t1], Z1[:], Tanh)
                if not last_blk:
                    nc.vector.tensor_mul(
                        AYP[:, :, TB:2 * TB, :], ASB[:, :, TB:2 * TB, :],
                        ybc(Y0, TB))
                    tree_reduce2(nc.vector, AYP, CT, "ax")

                nc.sync.dma_start(yo_e[:, :, t0:t1], Y[:, :, t0:t1])
                if last_blk:
                    break

                CC = CT[:, :, TB:2 * TB]     # cross contribution slice

                # -- A tensors for block blk+1 (PE + ScE early) --
                ASBn = make_asb(PRn)

                # -- hebb prefix: YTR copy, HT matmuls, AEFF --
                if blk >= 1:
                    nc.scalar.activation(YTR[:], YTP[:], Copy)
                    for ic in range(NG):
                        nc.tensor.matmul(
                            HT[:, ic, :],
                            XH[:, blk - 1, ic * 128:(ic + 1) * 128],
                            YTR[:, :, :],
                            start=(blk == 1), stop=(blk == NB - 2),
                            skip_group_check=True)
                    nc.scalar.activation(HTC[:], HT[:], Copy)
                    nc.vector.tensor_mul(AEFF[:], AB[:], HTC[:])

                # -- aeff part of base for block blk+1 --
                BB = None
                if blk >= 1:
                    BB = ps_bb.tile([128, NG, TB], f32, tag="bb")
                    k = 0
                    for jc in range(NG):
                        for ig in range(NG):
                            nc.tensor.matmul(
                                BB[:, jc, :],
                                AEFF[:, ig, jc * 128:(jc + 1) * 128],
                                XTD[:, ig, t1:t1 + TB],
                                start=(k == 0), stop=(k == NG * NG - 1),
                                skip_group_check=True)
                            k += 1

                # -- assemble Z0(blk+1) = WBASE + cross (+ BB) --
                CW = bp.tile([128, NG, TB], f32, tag="cw")
                nc.vector.scalar_tensor_tensor(
                    CW[:], CC, 1.0, WBASE[:, :, t1:t1 + TB], mult, add)
                if BB is not None:
                    Z0 = bp.tile([128, NG, TB], f32, tag="z0")
                    nc.vector.scalar_tensor_tensor(
                        Z0[:], BB[:], 1.0, CW[:], mult, add)
                    z_src = Z0[:]
                else:
                    z_src = CW[:]

                # prefetch pair data for block blk+2
                if blk + 2 < NB:
                    PRn = bp.tile([128, NG, 2 * TB, TB], bfd, tag="pr")
                    nc.sync.dma_start(PRn[:], pr_e[:, blk + 2])
                ASB = ASBn

    nc.compile()
    return nc


def kernel(x, w, alpha, eta, _trace=False, _trace_kwargs=None):
    from concourse.bass_utils import run_bass_kernel_spmd

    x = np.asarray(x, np.float32)
    w = np.asarray(w, np.float32)
    alpha = np.asarray(alpha, np.float32)
    eta_f = float(np.asarray(eta).reshape(-1)[0])

    d = 1.0 - eta_f
    t_idx = np.arange(T, dtype=np.float64)
    wscale = (d ** t_idx).astype(np.float32)                   # d^t
    qscale = (eta_f * d ** (-1.0 - t_idx)).astype(np.float32)  # eta*d^(-1-s)

    def to_grp(m, dt=BF16):  # [T,N] (cols=i) -> [128, NG, T], i = ig*128+ip
        return np.ascontiguousarray(
            m.T.reshape(NG, 128, T).transpose(1, 0, 2)).astype(dt)

    def to_wgrp(m, dt=BF16):  # [N,N] -> [128, NG, N], i = ig*128+ip
        return np.ascontiguousarray(
            m.reshape(NG, 128, N).transpose(1, 0, 2)).astype(dt)

    wm = to_wgrp(w)
    ab = to_wgrp(alpha)
    ident = np.eye(128, dtype=np.float32).astype(BF16)

    # pair tensor: pr[ip, b, ig, tl, s] = xtd[i, b*TB + tl] * xq[i, b*TB + s]
    # tl in [0,TB): in-block half, strictly-lower-masked (s < tl)
    # tl in [TB,2TB): cross half (t' = b*TB + TB + (tl-TB)), unmasked
    tril = np.tril(np.ones((TB, TB), np.float32), -1)

    in_maps = []
    for b_i in range(B):
        xb = x[b_i]                                 # [T, N]
        xtd = (xb * wscale[:, None]).astype(BF16).astype(np.float32)
        xq = (xb * qscale[:, None]).astype(BF16).astype(np.float32)
        pr = np.zeros((128, NB, NG, 2 * TB, TB), np.float32)
        for blk in range(NB):
            t0 = blk * TB
            xtd_in = xtd[t0:t0 + TB]                # [TB, N]
            xq_b = xq[t0:t0 + TB]                   # [TB, N]
            pin = np.einsum('tn,sn,ts->nts', xtd_in, xq_b, tril)
            g = pin.reshape(NG, 128, TB, TB).transpose(1, 0, 2, 3)
            pr[:, blk, :, 0:TB, :] = g
            if blk < NB - 1:
                xtd_c = xtd[t0 + TB:t0 + 2 * TB]
                pc = np.einsum('tn,sn->nts', xtd_c, xq_b)
                pr[:, blk, :, TB:2 * TB, :] = \
                    pc.reshape(NG, 128, TB, TB).transpose(1, 0, 2, 3)
        in_maps.append({
            "xt": to_grp(xb),
            "xtd": to_grp(xb * wscale[:, None]),
            "xh": np.ascontiguousarray(
                (xb * qscale[:, None]).reshape(NB, TB, N)
                .transpose(1, 0, 2)).astype(BF16),
            "wm": wm, "ab": ab, "ident": ident,
            "pr": np.ascontiguousarray(pr).astype(BF16),
        })

    nc = _build(eta_f)
    res = run_bass_kernel_spmd(
        nc, in_maps, list(range(N_CORES)),
        trace=_trace, **(_trace_kwargs or {}))

    out = np.empty((B, T, N), np.float32)
    for b_i in range(B):
        yo = res.results[b_i]["yout"]               # [128, NG, T] bf16
        out[b_i] = yo.astype(np.float32).transpose(2, 1, 0).reshape(T, N)
    if _trace:
        kernel.last_result = res
    return out

